# revision 2
# baseline (speedup 1.0000x reference)
"""nn_ConvSOM_dense1 Trainium2 kernel: 3x GCNConv + SOM scatter + dense head.

Self-contained: host prep (edge sort/pad, degree norm), Bass/Tile SPMD kernel
on 8 NeuronCores, host gather of per-core G partials + tiny final linear.

Sharding: nodes (and their in-edges) split contiguously across 8 cores;
GCN weights / SOM codebook replicated. Per conv: each core computes its
h=x@W shard, an AllGather collective replicates the bf16 message table,
edges are aggregated per 128-node tile via dma_gather (int16 indices into
half-table views) + one-hot matmuls. Per-graph G accumulators are summed
across cores on the host (the only cross-shard reduction), then the final
[64,256]@[256,1] linear + sigmoid runs on host.
"""
import numpy as np
import ml_dtypes

N = 50000
E = 800000
C = 128
P0, P1 = 16, 16
NUM_GRAPHS = 64
SIGMA = 2.0
NEG_SLOPE = 0.01
NC_ = 8
NPC = 6272            # nodes per core = 49*128
NSTAR = NC_ * NPC     # 50176
NT = NPC // 128       # 49 node tiles / core
HALF = NSTAR // 2     # 25088
NQ = 3                # SWDGE queues for dma_gather
GRP = 7               # node tiles per grouped gather (49 = 7*7)
P = 128

_CACHE = {}
TRACE = False
LAST_EXEC_NS = None


def _host_prep(x, edge_index, batch):
    src = np.asarray(edge_index[0], dtype=np.int64)
    dst = np.asarray(edge_index[1], dtype=np.int64)
    loops = np.arange(N, dtype=np.int64)
    s = np.concatenate([src, loops])
    d = np.concatenate([dst, loops])
    deg = np.bincount(d, minlength=N).astype(np.float32)
    dinv = np.where(deg > 0, deg ** -0.5, 0.0).astype(np.float32)
    norm = dinv[s] * dinv[d]

    core = d // NPC
    tloc = (d % NPC) // 128
    half = (s >= HALF).astype(np.int64)
    key = core * (NT * 2) + tloc * 2 + half
    counts = np.bincount(key, minlength=NC_ * NT * 2).reshape(NC_, NT, 2)
    T = np.maximum(np.ceil(counts.max(axis=0) / 128).astype(np.int64), 0)  # [NT,2]
    T[:, 0] = np.maximum(T[:, 0], 1)
    slot_sz = T * 128                                   # [NT,2]
    # segment order: for each group of GRP node tiles, half A tiles then half B
    seg_order = [(t, h) for g in range(NT // GRP)
                 for h in range(2) for t in range(g * GRP, (g + 1) * GRP)]
    seg_off = np.zeros((NT, 2), np.int64)
    acc = 0
    for (t, h) in seg_order:
        seg_off[t, h] = acc
        acc += int(slot_sz[t, h])
    nslots = acc                                        # per core
    n_et = nslots // 128

    order = np.argsort(key, kind="stable")
    sk, ss, sd, sn = key[order], s[order], d[order], norm[order]
    # rank within each (core,t,h) group
    grp_start = np.zeros(NC_ * NT * 2, np.int64)
    cnt_flat = counts.reshape(-1)
    grp_start[1:] = np.cumsum(cnt_flat)[:-1]
    rank = np.arange(len(sk)) - grp_start[sk]
    score = sk % (NT * 2)
    slot = seg_off.reshape(-1)[score] + rank            # within-core slot
    score_core = sk // (NT * 2)

    idx_all = np.zeros((NC_, nslots), np.int16)
    dl_all = np.full((NC_, nslots), -1.0, ml_dtypes.bfloat16)
    en_all = np.zeros((NC_, nslots), np.float32)
    lochalf = ss - (ss >= HALF) * HALF
    idx_all[score_core, slot] = lochalf.astype(np.int16)
    dl_all[score_core, slot] = (sd % 128).astype(np.float32)
    en_all[score_core, slot] = sn

    # idx16: per (t,h) segment column-major wrap over 16 partitions
    idx16 = np.zeros((NC_, 16, nslots // 16), np.int16)
    for t in range(NT):
        for h in range(2):
            o, sz = int(seg_off[t, h]), int(slot_sz[t, h])
            if sz == 0:
                continue
            seg = idx_all[:, o:o + sz]                  # [NC_, sz]
            idx16[:, :, o // 16:(o + sz) // 16] = \
                seg.reshape(NC_, sz // 16, 16).transpose(0, 2, 1)
    dl16 = dl_all.reshape(NC_, n_et, 128).transpose(0, 2, 1).copy()
    en16 = en_all.reshape(NC_, n_et, 128).transpose(0, 2, 1).copy()

    xpad = np.zeros((NSTAR, C), np.float32)
    xpad[:N] = np.asarray(x, np.float32)
    xT = xpad.reshape(NC_, NPC, C).transpose(0, 2, 1).copy()   # [NC_,128,6272]

    bpad = np.full(NSTAR, -1.0, np.float32)
    bpad[:N] = np.asarray(batch, np.float32)
    batch16 = bpad.reshape(NC_, NT, 128).transpose(0, 2, 1).copy()  # [NC_,128,49]

    return dict(T=T, n_et=n_et, idx16=idx16, dl16=dl16, en16=en16,
                enb16=en16.astype(ml_dtypes.bfloat16),
                xT=xT, batch16=batch16)


def _build(T, n_et):
    import concourse.bass as bass
    import concourse.bacc as bacc
    import concourse.tile as tile
    import concourse.mybir as mybir
    from concourse.library_config import mlp
    dt = mybir.dt
    AF = mybir.ActivationFunctionType
    OP = mybir.AluOpType
    INV2S2 = 1.0 / (2.0 * SIGMA * SIGMA)
    Stot = n_et * 8

    nc = bacc.Bacc("TRN2", target_bir_lowering=False, debug=False,
                   num_devices=NC_, num_swdge_queues=NQ)
    xT_d = nc.dram_tensor("xT", [P, NPC], dt.float32, kind="ExternalInput")
    idx_d = nc.dram_tensor("idx16", [16, Stot], dt.int16, kind="ExternalInput")
    dl_d = nc.dram_tensor("dl16", [P, n_et], dt.bfloat16, kind="ExternalInput")
    en_d = nc.dram_tensor("en16", [P, n_et], dt.float32, kind="ExternalInput")
    enb_d = nc.dram_tensor("enb16", [P, n_et], dt.bfloat16, kind="ExternalInput")
    bt_d = nc.dram_tensor("batch16", [P, NT], dt.float32, kind="ExternalInput")
    W_d = [nc.dram_tensor(f"W{k}", [C, C], dt.float32, kind="ExternalInput")
           for k in (1, 2, 3)]
    b_d = [nc.dram_tensor(f"b{k}", [C, 1], dt.float32, kind="ExternalInput")
           for k in (1, 2, 3)]
    sft_d = nc.dram_tensor("SfT", [3 * C, 256], dt.float32, kind="ExternalInput")
    g_out = nc.dram_tensor("g_out", [64, 256], dt.float32, kind="ExternalOutput")

    with tile.TileContext(nc) as tc:
        with (
            tc.tile_pool(name="cst", bufs=1) as cst,
            tc.tile_pool(name="xk", bufs=1) as xkp,
            tc.tile_pool(name="sb", bufs=4) as sb,
            tc.tile_pool(name="sb2", bufs=2) as sb2,
            tc.tile_pool(name="ps", bufs=3, space="PSUM") as ps,
            tc.tile_pool(name="pshs", bufs=2, space="PSUM") as pshs,
            tc.tile_pool(name="psg", bufs=1, space="PSUM") as psg,
            tc.tile_pool(name="dram", bufs=1, space="DRAM") as dram,
        ):
            nc.gpsimd.load_library(mlp)
            # ---- constant loads ----
            idx_sb = cst.tile([128, Stot], dt.int16)
            nc.vector.memset(idx_sb[:], 0)
            for q in range(NQ):
                nc.sync.dma_start(idx_sb[q * 32:q * 32 + 16, :], idx_d[:])
                nc.sync.dma_start(idx_sb[q * 32 + 16:q * 32 + 32, :], idx_d[:])
            dl_sb = cst.tile([P, n_et], dt.bfloat16)
            en_sb = cst.tile([P, n_et], dt.float32)
            enb_sb = cst.tile([P, n_et], dt.bfloat16)
            nc.sync.dma_start(enb_sb[:], enb_d[:])
            btl_sb = cst.tile([P, NT], dt.float32)
            nc.sync.dma_start(dl_sb[:], dl_d[:])
            nc.sync.dma_start(en_sb[:], en_d[:])
            nc.sync.dma_start(btl_sb[:], bt_d[:])
            W_sb, b_sb, sft_sb = [], [], []
            for k in range(3):
                w = cst.tile([C, C], dt.float32, tag=f"W{k}")
                nc.sync.dma_start(w[:], W_d[k][:])
                W_sb.append(w)
                b = cst.tile([C, 1], dt.float32, tag=f"b{k}")
                nc.sync.dma_start(b[:], b_d[k][:])
                b_sb.append(b)
                sft = cst.tile([C, 256], dt.float32, tag=f"sft{k}")
                nc.sync.dma_start(sft[:], sft_d[k * C:(k + 1) * C, :])
                sft_sb.append(sft)
            iota_b = cst.tile([P, P], dt.bfloat16)
            iota_i = cst.tile([P, P], dt.int32)
            nc.gpsimd.iota(iota_i[:], pattern=[[1, P]], base=0, channel_multiplier=0)
            nc.vector.tensor_copy(iota_b[:], iota_i[:])
            iota64 = cst.tile([P, 64], dt.float32)
            iota64_i = cst.tile([P, 64], dt.int32)
            nc.gpsimd.iota(iota64_i[:], pattern=[[1, 64]], base=0, channel_multiplier=0)
            nc.vector.tensor_copy(iota64[:], iota64_i[:])
            iota16 = cst.tile([P, 16], dt.float32)
            iota16_i = cst.tile([P, 16], dt.int32)
            nc.gpsimd.iota(iota16_i[:], pattern=[[1, 16]], base=0, channel_multiplier=0)
            nc.vector.tensor_copy(iota16[:], iota16_i[:])
            ones_col = cst.tile([P, 1], dt.float32)
            nc.vector.memset(ones_col[:], 1.0)
            ones_row = cst.tile([1, P], dt.float32)
            nc.vector.memset(ones_row[:], 1.0)

            # srow = -0.5*||S_u||^2  [1,256]
            srow_ps = psg.tile([1, 256], dt.float32, space="PSUM", tag="srow")
            for k in range(3):
                sq = sb.tile([C, 256], dt.float32, tag="ssq")
                nc.scalar.activation(sq[:], sft_sb[k][:], AF.Square)
                nc.tensor.matmul(srow_ps[:], lhsT=ones_col[:], rhs=sq[:],
                                 start=(k == 0), stop=(k == 2))
            srow = cst.tile([1, 256], dt.float32)
            nc.scalar.activation(srow[:], srow_ps[:], AF.Identity, scale=-0.5)

            x_cur = [xkp.tile([P, NPC], dt.float32, tag=f"x{k}", name=f"x{k}")
                     for k in range(3)]

            hn_shard = [dram.tile([NPC, C], dt.bfloat16, tag=f"hns{k}",
                                   name=f"hns{k}") for k in range(3)]
            hn_full = [dram.tile([NSTAR, C], dt.bfloat16, tag=f"hnf{k}",
                                  name=f"hnf{k}", addr_space="Shared")
                       for k in range(3)]

            gq = 0
            for k in range(3):
                # h-phase: h = x @ W  -> bf16 rows to hn_shard
                for t in range(NT):
                    if k == 0:
                        xchunk = sb.tile([P, 128], dt.float32, tag="xin")
                        nc.sync.dma_start(xchunk[:], xT_d[:, t * 128:(t + 1) * 128])
                        lhs_x = xchunk[:]
                    else:
                        lhs_x = x_cur[k - 1][:, t * 128:(t + 1) * 128]
                    h_ps = ps.tile([P, C], dt.float32, space="PSUM", tag="w")
                    nc.tensor.matmul(h_ps[:], lhsT=lhs_x,
                                     rhs=W_sb[k][:], start=True, stop=True)
                    h_bf = sb.tile([P, C], dt.bfloat16, tag="hbf")
                    nc.scalar.activation(h_bf[:], h_ps[:], AF.Identity)
                    nc.sync.dma_start(hn_shard[k][t * 128:(t + 1) * 128, :], h_bf[:])
                nc.gpsimd.collective_compute(
                    "AllGather", mybir.AluOpType.bypass,
                    replica_groups=[list(range(NC_))],
                    ins=[hn_shard[k].opt()], outs=[hn_full[k].opt()])
                # aggregation: grouped gathers (GRP node tiles x 2 halves)
                for grp in range(NT // GRP):
                    tiles = list(range(grp * GRP, (grp + 1) * GRP))
                    gbuf = {}
                    for h in range(2):
                        Tg = int(sum(T[t, h] for t in tiles))
                        if Tg == 0:
                            continue
                        o = int(_SEGOFF[tiles[0]][h])
                        gb = sb2.tile([P, _TGMAX, P], dt.bfloat16,
                                      tag=f"g{h}", name=f"g{h}")
                        view = hn_full[k][0:HALF, :] if h == 0 \
                            else hn_full[k][HALF:NSTAR, :]
                        nc.gpsimd.dma_gather(
                            out_ap=gb[:, :Tg, :], in_ap=view,
                            idxs_ap=idx_sb[:, o // 16:(o + Tg * 128) // 16],
                            num_idxs=Tg * 128, num_idxs_reg=Tg * 128,
                            elem_size=P, single_packet=False, queue_num=gq % NQ)
                        gq += 1
                        gbuf[h] = (gb, o)
                    for t in tiles:
                        agg_ps = ps.tile([C, P], dt.float32, space="PSUM", tag="w")
                        tot = int(T[t, 0] + T[t, 1])
                        cnt = 0
                        for h in range(2):
                            Tt = int(T[t, h])
                            if Tt == 0:
                                continue
                            gb, go = gbuf[h]
                            base = (int(_SEGOFF[t][h]) - go) // 128
                            for tt in range(Tt):
                                J = int(_SEGOFF[t][h]) // 128 + tt
                                oh = sb.tile([P, P], dt.bfloat16, tag="oh")
                                nc.vector.tensor_tensor(
                                    out=oh[:],
                                    in0=dl_sb[:, J:J + 1].to_broadcast([P, P]),
                                    in1=iota_b[:], op=OP.is_equal)
                                if cnt % 2 == 0:
                                    msc = sb.tile([P, P], dt.bfloat16, tag="msc")
                                    nc.scalar.activation(
                                        msc[:], gb[:, base + tt, :], AF.Identity,
                                        scale=en_sb[:, J:J + 1])
                                    rhs_m, lhs_m = oh, msc
                                else:
                                    ohw = sb.tile([P, P], dt.bfloat16, tag="ohw")
                                    nc.vector.tensor_tensor(
                                        out=ohw[:], in0=oh[:],
                                        in1=enb_sb[:, J:J + 1].to_broadcast([P, P]),
                                        op=OP.mult)
                                    rhs_m = ohw
                                    lhs_m = None
                                nc.tensor.matmul(
                                    agg_ps[:],
                                    lhsT=(lhs_m[:] if lhs_m is not None
                                          else gb[:, base + tt, :]),
                                    rhs=rhs_m[:],
                                    start=(cnt == 0), stop=(cnt == tot - 1))
                                cnt += 1
                        nc.scalar.activation(x_cur[k][:, t * 128:(t + 1) * 128],
                                             agg_ps[:], AF.Lrelu,
                                             bias=b_sb[k][:, :1], alpha=NEG_SLOPE)

            # ---- SOM phase ----
            G_ps = psg.tile([64, 256], dt.float32, space="PSUM", tag="G")
            import dataclasses as _dc
            for t in range(NT):
                sl = slice(t * 128, (t + 1) * 128)
                D_ps = ps.tile([P, 256], dt.float32, space="PSUM", tag="w")
                hs_ps = pshs.tile([P, 1], dt.float32, space="PSUM", tag="hs")
                for k in range(3):
                    nc.tensor.matmul(D_ps[:], lhsT=x_cur[k][:, sl], rhs=sft_sb[k][:],
                                     start=(k == 0), stop=False,
                                     skip_group_check=True)
                    sq = sb.tile([P, P], dt.float32, tag="xsq")
                    nc.scalar.activation(sq[:], x_cur[k][:, sl], AF.Square)
                    nc.tensor.matmul(hs_ps[:], lhsT=sq[:], rhs=ones_col[:],
                                     start=(k == 0), stop=(k == 2),
                                     skip_group_check=True)
                nc.tensor.matmul(D_ps[:], lhsT=ones_row[:], rhs=srow[:],
                                 start=False, stop=True,
                                 skip_group_check=True)
                mx = sb.tile([P, 8], dt.float32, tag="mx")
                mi = sb.tile([P, 8], dt.uint32, tag="mi")
                nc.vector.max_with_indices(mx[:], mi[:], D_ps[:])
                wj_u = sb.tile([P, 1], dt.uint32, tag="wju")
                nc.vector.tensor_scalar(out=wj_u[:], in0=mi[:, :1], scalar1=15,
                                        scalar2=None, op0=OP.bitwise_and)
                wi_u = sb.tile([P, 1], dt.uint32, tag="wiu")
                nc.vector.tensor_scalar(out=wi_u[:], in0=mi[:, :1], scalar1=4,
                                        scalar2=None, op0=OP.logical_shift_right)
                nwj = sb.tile([P, 1], dt.float32, tag="nwj")
                nwi = sb.tile([P, 1], dt.float32, tag="nwi")
                wj_f = sb.tile([P, 1], dt.float32, tag="wjf")
                wi_f = sb.tile([P, 1], dt.float32, tag="wif")
                nc.vector.tensor_copy(wj_f[:], wj_u[:])
                nc.vector.tensor_copy(wi_f[:], wi_u[:])
                nc.scalar.activation(nwj[:], wj_f[:], AF.Identity, scale=-1.0)
                nc.scalar.activation(nwi[:], wi_f[:], AF.Identity, scale=-1.0)
                # d2min = hsum - 2*max  (clamped)
                m2 = sb.tile([P, 1], dt.float32, tag="m2")
                nc.scalar.activation(m2[:], mx[:, :1], AF.Identity, scale=-2.0)
                d2 = sb.tile([P, 1], dt.float32, tag="d2")
                nc.vector.tensor_add(d2[:], m2[:], hs_ps[:])
                nc.vector.tensor_scalar_max(d2[:], d2[:], 0.0)
                mind = sb.tile([P, 1], dt.float32, tag="mind")
                nc.scalar.activation(mind[:], d2[:], AF.Sqrt)
                hsv = sb.tile([P, 1], dt.float32, tag="hsv")
                nc.scalar.activation(hsv[:], mind[:], AF.Exp, scale=-1.0)
                ax = sb.tile([P, 16], dt.float32, tag="ax")
                nc.scalar.activation(ax[:], iota16[:], AF.Square, bias=nwi[:, :1])
                nc.scalar.activation(ax[:], ax[:], AF.Exp, scale=-INV2S2)
                nc.scalar.activation(ax[:], ax[:], AF.Identity, scale=hsv[:, :1])
                ay = sb.tile([P, 16], dt.float32, tag="ay")
                nc.scalar.activation(ay[:], iota16[:], AF.Square, bias=nwj[:, :1])
                nc.scalar.activation(ay[:], ay[:], AF.Exp, scale=-INV2S2)
                ax_ap = ax[:]
                ax_b = _dc.replace(ax_ap, ap=[ax_ap.ap[0], ax_ap.ap[1], [0, 16]])
                ay_ap = ay[:]
                ay_b = _dc.replace(ay_ap, ap=[ay_ap.ap[0], [0, 16], ay_ap.ap[1]])
                contrib = sb2.tile([P, 256], dt.float32, tag="contrib")
                nc.vector.tensor_tensor(out=contrib[:], in0=ax_b, in1=ay_b,
                                        op=OP.mult)
                bt = sb.tile([P, 64], dt.float32, tag="bt")
                nc.vector.tensor_tensor(
                    out=bt[:], in0=btl_sb[:, t:t + 1].to_broadcast([P, 64]),
                    in1=iota64[:], op=OP.is_equal)
                nc.tensor.matmul(G_ps[:], lhsT=bt[:], rhs=contrib[:],
                                 start=(t == 0), stop=(t == NT - 1),
                                 skip_group_check=True)
            G_sb = cst.tile([64, 256], dt.float32)
            nc.scalar.activation(G_sb[:], G_ps[:], AF.Identity)
            nc.sync.dma_start(g_out[:], G_sb[:])
    nc.compile()
    return nc


_SEGOFF = None
_TMAX = None
_TGMAX = None


def kernel(**inputs):
    global _SEGOFF, _TMAX, _TGMAX
    from concourse.bass_utils import run_bass_kernel_spmd

    x = np.asarray(inputs["x"], np.float32)
    prep = _host_prep(x, np.asarray(inputs["edge_index"]),
                      np.asarray(inputs["batch"]))
    T, n_et = prep["T"], prep["n_et"]
    seg_order = [(t, h) for g in range(NT // GRP)
                 for h in range(2) for t in range(g * GRP, (g + 1) * GRP)]
    seg_off = np.zeros((NT, 2), np.int64)
    acc = 0
    for (t, h) in seg_order:
        seg_off[t, h] = acc
        acc += int(T[t, h]) * 128
    _SEGOFF = seg_off
    _TMAX = int(T.max())
    _TGMAX = int(max(T[g * GRP:(g + 1) * GRP, h].sum()
                     for g in range(NT // GRP) for h in range(2)))

    ck = (n_et, tuple(T.reshape(-1).tolist()))
    if ck not in _CACHE:
        _CACHE[ck] = _build(T, n_et)
    nc = _CACHE[ck]

    SfT = np.asarray(inputs["S"], np.float32).reshape(256, 384).T.copy()
    in_maps = []
    for c in range(NC_):
        m = dict(
            xT=prep["xT"][c], idx16=prep["idx16"][c], dl16=prep["dl16"][c],
            en16=prep["en16"][c], enb16=prep["enb16"][c],
            batch16=prep["batch16"][c],
            SfT=SfT,
            W1=np.asarray(inputs["W1"], np.float32),
            W2=np.asarray(inputs["W2"], np.float32),
            W3=np.asarray(inputs["W3"], np.float32),
            b1=np.asarray(inputs["b1"], np.float32).reshape(C, 1),
            b2=np.asarray(inputs["b2"], np.float32).reshape(C, 1),
            b3=np.asarray(inputs["b3"], np.float32).reshape(C, 1),
        )
        in_maps.append(m)
    global LAST_EXEC_NS
    kw = {}
    if TRACE:
        try:
            import trn_prof
            trn_prof.install()
            kw = dict(trace=True, tmpdir="/tmp/ktrace")
            import os as _os, shutil as _sh
            _sh.rmtree("/tmp/ktrace", ignore_errors=True)
            _os.makedirs("/tmp/ktrace", exist_ok=True)
        except Exception:
            pass
    res = run_bass_kernel_spmd(nc, in_maps, core_ids=list(range(NC_)), **kw)
    LAST_EXEC_NS = res.exec_time_ns
    G = np.zeros((64, 256), np.float64)
    for c in range(NC_):
        G += res.results[c]["g_out"].astype(np.float64)
    lin_W = np.asarray(inputs["lin_W"], np.float32)
    lin_b = np.asarray(inputs["lin_b"], np.float32)
    z = G.astype(np.float32) @ lin_W.T + lin_b
    return (1.0 / (1.0 + np.exp(-z))).astype(np.float32)



# revision 8
# speedup vs baseline: 1.1091x; 1.1091x over previous
"""nn_ConvSOM_dense1 Trainium2 kernel: 3x GCNConv + SOM scatter + dense head.

Self-contained: host prep (edge sort/pad, degree norm), Bass/Tile SPMD kernel
on 8 NeuronCores, host gather of per-core G partials + tiny final linear.

v2 design (vs baseline):
- 4 SWDGE queues (max), gathers round-robin + deep buffering so the
  descriptor drain (~2ns/row aggregate) overlaps compute.
- Message table in bf16; weighted one-hot built in ONE DVE dual-op
  instruction per edge tile: (iota == dl) * norm.
- Raw gathered rows feed the scatter matmul directly (no per-tile scalar
  scale op).
- AllGather split in two (shard halves re-keyed) and pipelined: conv k+1's
  first-half AllGather is issued between conv k's gather batches.
- SOM phase inlined per destination tile into conv 3's aggregation.
"""
import numpy as np
import ml_dtypes

N = 50000
E = 800000
C = 128
P0, P1 = 16, 16
NUM_GRAPHS = 64
SIGMA = 2.0
NEG_SLOPE = 0.01
NC_ = 8
NPC = 6272            # nodes per core = 49*128
NSTAR = NC_ * NPC     # 50176
NT = NPC // 128       # 49 node tiles / core
HS0 = 3200            # first-half shard rows (25 tiles)
HS1 = NPC - HS0       # 3072 (24 tiles)
T0 = NC_ * HS0        # 25600 rows in half-table 0
T1 = NC_ * HS1        # 24576 rows in half-table 1
NQ = 4                # SWDGE queues
P = 128

GROUPS = [list(range(0, 5)), list(range(5, 10)), list(range(10, 15)),
          list(range(15, 20)), list(range(20, 25)),
          list(range(25, 29)), list(range(29, 33)), list(range(33, 37)),
          list(range(37, 41)), list(range(41, 45)), list(range(45, 49))]
NG1 = 5  # groups covering tiles 0-24
# emission phases: (groups, half)
PHASES = [(GROUPS[:NG1], 0), (GROUPS[:NG1], 1),
          (GROUPS[NG1:], 0), (GROUPS[NG1:], 1)]


def _seg_order():
    order = []
    for grps, h in PHASES:
        for g in grps:
            for t in g:
                order.append((t, h))
    return order


_CACHE = {}
TRACE = False
LAST_EXEC_NS = None
LAST_G = None


def _host_prep(x, edge_index, batch):
    src = np.asarray(edge_index[0], dtype=np.int64)
    dst = np.asarray(edge_index[1], dtype=np.int64)
    loops = np.arange(N, dtype=np.int64)
    s = np.concatenate([src, loops])
    d = np.concatenate([dst, loops])
    deg = np.bincount(d, minlength=N).astype(np.float32)
    dinv = np.where(deg > 0, deg ** -0.5, 0.0).astype(np.float32)
    norm = dinv[s] * dinv[d]

    core = d // NPC
    tloc = (d % NPC) // 128
    r = s % NPC
    half = (r >= HS0).astype(np.int64)
    key = core * (NT * 2) + tloc * 2 + half
    counts = np.bincount(key, minlength=NC_ * NT * 2).reshape(NC_, NT, 2)
    T = np.ceil(counts.max(axis=0) / 128).astype(np.int64)  # [NT,2]
    seg_order = _seg_order()
    seg_off = np.zeros((NT, 2), np.int64)
    acc = 0
    for (t, h) in seg_order:
        seg_off[t, h] = acc
        acc += int(T[t, h]) * 128
    nslots = acc
    n_et = nslots // 128

    order = np.argsort(key, kind="stable")
    sk, ss, sd, sn = key[order], s[order], d[order], norm[order]
    grp_start = np.zeros(NC_ * NT * 2, np.int64)
    cnt_flat = counts.reshape(-1)
    grp_start[1:] = np.cumsum(cnt_flat)[:-1]
    rank = np.arange(len(sk)) - grp_start[sk]
    score = sk % (NT * 2)
    slot = seg_off.reshape(-1)[score] + rank
    score_core = sk // (NT * 2)

    idx_all = np.zeros((NC_, nslots), np.int16)
    dl_all = np.full((NC_, nslots), -1.0, np.float32)
    en_all = np.zeros((NC_, nslots), np.float32)
    sc = ss // NPC
    sr = ss % NPC
    lochalf = np.where(sr < HS0, sc * HS0 + sr, sc * HS1 + (sr - HS0))
    idx_all[score_core, slot] = lochalf.astype(np.int16)
    dl_all[score_core, slot] = (sd % 128).astype(np.float32)
    en_all[score_core, slot] = sn

    # idx16: per (t,h) segment column-major wrap over 16 partitions
    idx16 = np.zeros((NC_, 16, nslots // 16), np.int16)
    for t in range(NT):
        for h in range(2):
            o, sz = int(seg_off[t, h]), int(T[t, h]) * 128
            if sz == 0:
                continue
            seg = idx_all[:, o:o + sz]
            idx16[:, :, o // 16:(o + sz) // 16] = \
                seg.reshape(NC_, sz // 16, 16).transpose(0, 2, 1)
    dl16 = dl_all.reshape(NC_, n_et, 128).transpose(0, 2, 1).copy()
    en16 = en_all.reshape(NC_, n_et, 128).transpose(0, 2, 1).copy()

    xpad = np.zeros((NSTAR, C), np.float32)
    xpad[:N] = np.asarray(x, np.float32)
    xT = xpad.reshape(NC_, NPC, C).transpose(0, 2, 1)
    xT = np.ascontiguousarray(xT).astype(ml_dtypes.bfloat16)  # [NC_,128,NPC]

    bpad = np.full(NSTAR, -1.0, np.float32)
    bpad[:N] = np.asarray(batch, np.float32)
    batch16 = bpad.reshape(NC_, NT, 128).transpose(0, 2, 1).copy()  # [NC_,128,NT]

    return dict(T=T, n_et=n_et, idx16=idx16, dl16=dl16, en16=en16,
                xT=xT, batch16=batch16)


def _build(T, n_et):
    import concourse.bass as bass
    import concourse.bacc as bacc
    import concourse.tile as tile
    import concourse.mybir as mybir
    from concourse.library_config import mlp
    import dataclasses as _dc
    dt = mybir.dt
    AF = mybir.ActivationFunctionType
    OP = mybir.AluOpType
    INV2S2 = 1.0 / (2.0 * SIGMA * SIGMA)
    Stot = n_et * 8  # idx cols (nslots/16)

    seg_off = np.zeros((NT, 2), np.int64)
    acc = 0
    for (t, h) in _seg_order():
        seg_off[t, h] = acc
        acc += int(T[t, h]) * 128
    TGMAX = int(max(sum(int(T[t, h]) for t in g) for g in GROUPS for h in (0, 1)))
    TGMAX = max(TGMAX, 1)

    nc = bacc.Bacc("TRN2", target_bir_lowering=False, debug=False,
                   num_devices=NC_, num_swdge_queues=NQ)
    xT_d = nc.dram_tensor("xT", [P, NPC], dt.bfloat16, kind="ExternalInput")
    idx_d = nc.dram_tensor("idx16", [16, Stot], dt.int16, kind="ExternalInput")
    dl_d = nc.dram_tensor("dl16", [P, n_et], dt.float32, kind="ExternalInput")
    en_d = nc.dram_tensor("en16", [P, n_et], dt.float32, kind="ExternalInput")
    bt_d = nc.dram_tensor("batch16", [P, NT], dt.float32, kind="ExternalInput")
    W_d = [nc.dram_tensor(f"W{k}", [C, C], dt.bfloat16, kind="ExternalInput")
           for k in (1, 2, 3)]
    b_d = [nc.dram_tensor(f"b{k}", [C, 1], dt.float32, kind="ExternalInput")
           for k in (1, 2, 3)]
    sft_d = nc.dram_tensor("SfT", [3 * C, 256], dt.bfloat16, kind="ExternalInput")
    g_out = nc.dram_tensor("g_out", [64, 256], dt.float32, kind="ExternalOutput")

    with tile.TileContext(nc) as tc:
        with (
            tc.tile_pool(name="cst", bufs=1) as cst,
            tc.tile_pool(name="xk", bufs=1) as xkp,
            tc.tile_pool(name="sb", bufs=4) as sb,
            tc.tile_pool(name="gbuf", bufs=8) as gbp,
            tc.tile_pool(name="ps", bufs=3, space="PSUM") as ps,
            tc.tile_pool(name="pshs", bufs=2, space="PSUM") as pshs,
            tc.tile_pool(name="psg", bufs=1, space="PSUM") as psg,
            tc.tile_pool(name="dram", bufs=1, space="DRAM") as dram,
        ):
            nc.gpsimd.load_library(mlp)
            # ---- constant loads ----
            idx_sb = cst.tile([128, Stot], dt.int16)
            for q in range(NQ):
                nc.sync.dma_start(idx_sb[q * 32:q * 32 + 16, :], idx_d[:])
                nc.sync.dma_start(idx_sb[q * 32 + 16:q * 32 + 32, :], idx_d[:])
            dl_sb = cst.tile([P, n_et], dt.float32)
            en_sb = cst.tile([P, n_et], dt.float32)
            btl_sb = cst.tile([P, NT], dt.float32)
            nc.sync.dma_start(dl_sb[:], dl_d[:])
            nc.sync.dma_start(en_sb[:], en_d[:])
            nc.sync.dma_start(btl_sb[:], bt_d[:])
            W_sb, b_sb, sft_sb = [], [], []
            for k in range(3):
                w = cst.tile([C, C], dt.bfloat16, tag=f"W{k}")
                nc.sync.dma_start(w[:], W_d[k][:])
                W_sb.append(w)
                b = cst.tile([C, 1], dt.float32, tag=f"b{k}")
                nc.sync.dma_start(b[:], b_d[k][:])
                b_sb.append(b)
                sft = cst.tile([C, 256], dt.bfloat16, tag=f"sft{k}")
                nc.sync.dma_start(sft[:], sft_d[k * C:(k + 1) * C, :])
                sft_sb.append(sft)
            iota_b = cst.tile([P, P], dt.bfloat16)
            iota_i = cst.tile([P, P], dt.int32)
            nc.gpsimd.iota(iota_i[:], pattern=[[1, P]], base=0, channel_multiplier=0)
            nc.vector.tensor_copy(iota_b[:], iota_i[:])
            iota64 = cst.tile([P, 64], dt.float32)
            iota64_i = cst.tile([P, 64], dt.int32)
            nc.gpsimd.iota(iota64_i[:], pattern=[[1, 64]], base=0, channel_multiplier=0)
            nc.vector.tensor_copy(iota64[:], iota64_i[:])
            iota16 = cst.tile([P, 16], dt.float32)
            iota16_i = cst.tile([P, 16], dt.int32)
            nc.gpsimd.iota(iota16_i[:], pattern=[[1, 16]], base=0, channel_multiplier=0)
            nc.vector.tensor_copy(iota16[:], iota16_i[:])
            ones_col = cst.tile([P, 1], dt.float32)
            nc.vector.memset(ones_col[:], 1.0)
            ones_row = cst.tile([1, P], dt.float32)
            nc.vector.memset(ones_row[:], 1.0)

            # srow = -0.5*||S_u||^2  [1,256]
            srow_ps = psg.tile([1, 256], dt.float32, space="PSUM", tag="srow")
            for k in range(3):
                sq = sb.tile([C, 256], dt.float32, tag="ssq")
                nc.scalar.activation(sq[:], sft_sb[k][:], AF.Square)
                nc.tensor.matmul(srow_ps[:], lhsT=ones_col[:], rhs=sq[:],
                                 start=(k == 0), stop=(k == 2))
            srow = cst.tile([1, 256], dt.float32)
            nc.scalar.activation(srow[:], srow_ps[:], AF.Identity, scale=-0.5)

            x_cur = [xkp.tile([P, NPC], dt.bfloat16, tag=f"x{k}", name=f"x{k}")
                     for k in range(3)]

            hn_shard = [dram.tile([NPC, C], dt.bfloat16, tag=f"hns{k}",
                                  name=f"hns{k}") for k in range(3)]
            hn_f0 = [dram.tile([T0, C], dt.bfloat16, tag=f"hf0_{k}",
                               name=f"hf0_{k}", addr_space="Shared")
                     for k in range(3)]
            hn_f1 = [dram.tile([T1, C], dt.bfloat16, tag=f"hf1_{k}",
                               name=f"hf1_{k}", addr_space="Shared")
                     for k in range(3)]

            G_ps = psg.tile([64, 256], dt.float32, space="PSUM", tag="G")
            gq = [0]

            def h_tiles(k, t_lo, t_hi):
                # h = x @ W for shard tiles [t_lo, t_hi); write bf16 rows
                for t in range(t_lo, t_hi):
                    if k == 0:
                        xchunk = sb.tile([P, 128], dt.bfloat16, tag="xin")
                        nc.sync.dma_start(xchunk[:], xT_d[:, t * 128:(t + 1) * 128])
                        lhs_x = xchunk[:]
                    else:
                        lhs_x = x_cur[k - 1][:, t * 128:(t + 1) * 128]
                    h_ps = ps.tile([P, C], dt.float32, space="PSUM", tag="w")
                    nc.tensor.matmul(h_ps[:], lhsT=lhs_x,
                                     rhs=W_sb[k][:], start=True, stop=True)
                    h_bf = sb.tile([P, C], dt.bfloat16, tag="hbf")
                    nc.scalar.activation(h_bf[:], h_ps[:], AF.Identity)
                    nc.sync.dma_start(hn_shard[k][t * 128:(t + 1) * 128, :], h_bf[:])

            def allgather(k, h):
                if h == 0:
                    nc.gpsimd.collective_compute(
                        "AllGather", mybir.AluOpType.bypass,
                        replica_groups=[list(range(NC_))],
                        ins=[hn_shard[k][0:HS0, :].opt()], outs=[hn_f0[k].opt()])
                else:
                    nc.gpsimd.collective_compute(
                        "AllGather", mybir.AluOpType.bypass,
                        replica_groups=[list(range(NC_))],
                        ins=[hn_shard[k][HS0:NPC, :].opt()], outs=[hn_f1[k].opt()])

            def som_tile(t):
                sl = slice(t * 128, (t + 1) * 128)
                D_ps = ps.tile([P, 256], dt.float32, space="PSUM", tag="w")
                hs_ps = pshs.tile([P, 1], dt.float32, space="PSUM", tag="hs")
                for k in range(3):
                    nc.tensor.matmul(D_ps[:], lhsT=x_cur[k][:, sl], rhs=sft_sb[k][:],
                                     start=(k == 0), stop=False,
                                     skip_group_check=True)
                    sq = sb.tile([P, P], dt.float32, tag="xsq")
                    nc.scalar.activation(sq[:], x_cur[k][:, sl], AF.Square)
                    nc.tensor.matmul(hs_ps[:], lhsT=sq[:], rhs=ones_col[:],
                                     start=(k == 0), stop=(k == 2),
                                     skip_group_check=True)
                nc.tensor.matmul(D_ps[:], lhsT=ones_row[:], rhs=srow[:],
                                 start=False, stop=True,
                                 skip_group_check=True)
                mx = sb.tile([P, 8], dt.float32, tag="mx")
                mi = sb.tile([P, 8], dt.uint32, tag="mi")
                nc.vector.max_with_indices(mx[:], mi[:], D_ps[:])
                wj_u = sb.tile([P, 1], dt.uint32, tag="wju")
                nc.vector.tensor_scalar(out=wj_u[:], in0=mi[:, :1], scalar1=15,
                                        scalar2=None, op0=OP.bitwise_and)
                wi_u = sb.tile([P, 1], dt.uint32, tag="wiu")
                nc.vector.tensor_scalar(out=wi_u[:], in0=mi[:, :1], scalar1=4,
                                        scalar2=None, op0=OP.logical_shift_right)
                nwj = sb.tile([P, 1], dt.float32, tag="nwj")
                nwi = sb.tile([P, 1], dt.float32, tag="nwi")
                wj_f = sb.tile([P, 1], dt.float32, tag="wjf")
                wi_f = sb.tile([P, 1], dt.float32, tag="wif")
                nc.vector.tensor_copy(wj_f[:], wj_u[:])
                nc.vector.tensor_copy(wi_f[:], wi_u[:])
                nc.scalar.activation(nwj[:], wj_f[:], AF.Identity, scale=-1.0)
                nc.scalar.activation(nwi[:], wi_f[:], AF.Identity, scale=-1.0)
                m2 = sb.tile([P, 1], dt.float32, tag="m2")
                nc.scalar.activation(m2[:], mx[:, :1], AF.Identity, scale=-2.0)
                d2 = sb.tile([P, 1], dt.float32, tag="d2")
                nc.vector.tensor_add(d2[:], m2[:], hs_ps[:])
                nc.vector.tensor_scalar_max(d2[:], d2[:], 0.0)
                mind = sb.tile([P, 1], dt.float32, tag="mind")
                nc.scalar.activation(mind[:], d2[:], AF.Sqrt)
                hsv = sb.tile([P, 1], dt.float32, tag="hsv")
                nc.scalar.activation(hsv[:], mind[:], AF.Exp, scale=-1.0)
                ax = sb.tile([P, 16], dt.float32, tag="ax")
                nc.scalar.activation(ax[:], iota16[:], AF.Square, bias=nwi[:, :1])
                nc.scalar.activation(ax[:], ax[:], AF.Exp, scale=-INV2S2)
                nc.scalar.activation(ax[:], ax[:], AF.Identity, scale=hsv[:, :1])
                ay = sb.tile([P, 16], dt.float32, tag="ay")
                nc.scalar.activation(ay[:], iota16[:], AF.Square, bias=nwj[:, :1])
                nc.scalar.activation(ay[:], ay[:], AF.Exp, scale=-INV2S2)
                ax_ap = ax[:]
                ax_b = _dc.replace(ax_ap, ap=[ax_ap.ap[0], ax_ap.ap[1], [0, 16]])
                ay_ap = ay[:]
                ay_b = _dc.replace(ay_ap, ap=[ay_ap.ap[0], [0, 16], ay_ap.ap[1]])
                contrib = sb.tile([P, 256], dt.float32, tag="contrib")
                nc.vector.tensor_tensor(out=contrib[:], in0=ax_b, in1=ay_b,
                                        op=OP.mult)
                bt = sb.tile([P, 64], dt.float32, tag="bt")
                nc.vector.tensor_tensor(
                    out=bt[:], in0=btl_sb[:, t:t + 1].to_broadcast([P, 64]),
                    in1=iota64[:], op=OP.is_equal)
                nc.tensor.matmul(G_ps[:], lhsT=bt[:], rhs=contrib[:],
                                 start=(t == 0), stop=(t == NT - 1),
                                 skip_group_check=True)

            def agg_phase(k, grps, h):
                view = hn_f0[k][:] if h == 0 else hn_f1[k][:]
                gbuf = {}
                for gi, g in enumerate(grps):
                    Tg = int(sum(T[t, h] for t in g))
                    if Tg == 0:
                        gbuf[g[0]] = (None, 0)
                        continue
                    o = int(seg_off[g[0], h])
                    gb = gbp.tile([P, TGMAX, P], dt.bfloat16, tag="gb", name="gb")
                    nc.gpsimd.dma_gather(
                        out_ap=gb[:, :Tg, :], in_ap=view,
                        idxs_ap=idx_sb[:, o // 16:(o + Tg * 128) // 16],
                        num_idxs=Tg * 128, num_idxs_reg=Tg * 128,
                        elem_size=P, single_packet=False,
                        queue_num=gq[0] % NQ)
                    gq[0] += 1
                    gbuf[g[0]] = (gb, o)
                return gbuf

            def agg_group(k, g, gbuf0, gbuf1):
                # accumulate BOTH halves per dst tile, then activate (+SOM)
                gb0, go0 = gbuf0[g[0]]
                gb1, go1 = gbuf1[g[0]]
                for t in g:
                    agg_ps = ps.tile([C, P], dt.float32, space="PSUM", tag="w")
                    tot = int(T[t, 0] + T[t, 1])
                    cnt = 0
                    for h, gb, go in ((0, gb0, go0), (1, gb1, go1)):
                        Tt = int(T[t, h])
                        base = (int(seg_off[t, h]) - go) // 128
                        for tt in range(Tt):
                            J = int(seg_off[t, h]) // 128 + tt
                            ohw = sb.tile([P, P], dt.bfloat16, tag="ohw")
                            nc.vector.tensor_scalar(
                                out=ohw[:], in0=iota_b[:],
                                scalar1=dl_sb[:, J:J + 1],
                                scalar2=en_sb[:, J:J + 1],
                                op0=OP.is_equal, op1=OP.mult)
                            nc.tensor.matmul(
                                agg_ps[:], lhsT=gb[:, base + tt, :], rhs=ohw[:],
                                start=(cnt == 0), stop=(cnt == tot - 1),
                                skip_group_check=True)
                            cnt += 1
                    nc.scalar.activation(x_cur[k][:, t * 128:(t + 1) * 128],
                                         agg_ps[:], AF.Lrelu,
                                         bias=b_sb[k][:, :1], alpha=NEG_SLOPE)
                    if k == 2:
                        som_tile(t)

            def conv_half(k, grps):
                gbuf0 = agg_phase(k, grps, 0)
                gbuf1 = agg_phase(k, grps, 1)
                for g in grps:
                    agg_group(k, g, gbuf0, gbuf1)

            # ---------------- schedule ----------------
            h_tiles(0, 0, 25)
            allgather(0, 0)
            h_tiles(0, 25, NT)
            allgather(0, 1)
            for k in range(3):
                conv_half(k, GROUPS[:NG1])
                if k < 2:
                    h_tiles(k + 1, 0, 25)
                    allgather(k + 1, 0)
                conv_half(k, GROUPS[NG1:])
                if k < 2:
                    h_tiles(k + 1, 25, NT)
                    allgather(k + 1, 1)

            G_sb = cst.tile([64, 256], dt.float32)
            nc.scalar.activation(G_sb[:], G_ps[:], AF.Identity)
            nc.sync.dma_start(g_out[:], G_sb[:])
    nc.compile()
    return nc


def kernel(**inputs):
    from concourse.bass_utils import run_bass_kernel_spmd

    x = np.asarray(inputs["x"], np.float32)
    prep = _host_prep(x, np.asarray(inputs["edge_index"]),
                      np.asarray(inputs["batch"]))
    T, n_et = prep["T"], prep["n_et"]

    ck = (n_et, tuple(T.reshape(-1).tolist()))
    if ck not in _CACHE:
        _CACHE[ck] = _build(T, n_et)
    nc = _CACHE[ck]

    SfT = np.asarray(inputs["S"], np.float32).reshape(256, 384).T.copy()
    in_maps = []
    for c in range(NC_):
        m = dict(
            xT=prep["xT"][c], idx16=prep["idx16"][c], dl16=prep["dl16"][c],
            en16=prep["en16"][c],
            batch16=prep["batch16"][c],
            SfT=SfT.astype(ml_dtypes.bfloat16),
            W1=np.asarray(inputs["W1"], np.float32).astype(ml_dtypes.bfloat16),
            W2=np.asarray(inputs["W2"], np.float32).astype(ml_dtypes.bfloat16),
            W3=np.asarray(inputs["W3"], np.float32).astype(ml_dtypes.bfloat16),
            b1=np.asarray(inputs["b1"], np.float32).reshape(C, 1),
            b2=np.asarray(inputs["b2"], np.float32).reshape(C, 1),
            b3=np.asarray(inputs["b3"], np.float32).reshape(C, 1),
        )
        in_maps.append(m)
    global LAST_EXEC_NS, LAST_G
    kw = {}
    if TRACE:
        try:
            import trn_prof
            trn_prof.install()
            kw = dict(trace=True, tmpdir="/tmp/ktrace")
            import os as _os, shutil as _sh
            _sh.rmtree("/tmp/ktrace", ignore_errors=True)
            _os.makedirs("/tmp/ktrace", exist_ok=True)
        except Exception:
            pass
    res = run_bass_kernel_spmd(nc, in_maps, core_ids=list(range(NC_)), **kw)
    LAST_EXEC_NS = res.exec_time_ns
    G = np.zeros((64, 256), np.float64)
    for c in range(NC_):
        G += res.results[c]["g_out"].astype(np.float64)
    LAST_G = G.copy()
    lin_W = np.asarray(inputs["lin_W"], np.float32)
    lin_b = np.asarray(inputs["lin_b"], np.float32)
    z = G.astype(np.float32) @ lin_W.T + lin_b
    return (1.0 / (1.0 + np.exp(-z))).astype(np.float32)
